# revision 1
# baseline (speedup 1.0000x reference)
import sys

sys.path.insert(0, "/opt/trn_rl_repo")
import numpy as np

import concourse.bass as bass
import concourse.tile as tile
from concourse import bacc, mybir
from concourse.bass_utils import run_bass_kernel_spmd
from concourse.masks import make_identity

f32 = mybir.dt.float32
bf16 = mybir.dt.bfloat16
fp16 = mybir.dt.float16
u32 = mybir.dt.uint32
Exp = mybir.ActivationFunctionType.Exp
AX = mybir.AxisListType.X
MAX = mybir.AluOpType.max

B, N, D = 4, 4096, 64
NCORES = 8
NQ = 2048  # queries per core (half a batch)
NK = 4096  # keys per core
QT = NQ // 128  # 16 q-tiles
CH = NK // 128  # 32 k-chunks
BLK = 4  # query blocks of 512 (legacy)
BLOCKS = [(0, 1), (1, 1)] + [(2 + 2 * i, 2) for i in range(6)] + [(14, 1), (15, 1)]  # (tile0, ntiles)
SCALE = 64.0  # sqrt(N)

_cached = {}


def build_program():
    nc = bacc.Bacc("TRN2", target_bir_lowering=False, debug=False, num_devices=NCORES)
    q_d = nc.dram_tensor("q", [NQ, D], f32, kind="ExternalInput").ap()
    k_d = nc.dram_tensor("k", [NK, D], f32, kind="ExternalInput").ap()
    v_d = nc.dram_tensor("v", [NK, D], f32, kind="ExternalInput").ap()
    o_d = nc.dram_tensor("o", [NQ, D], f32, kind="ExternalOutput").ap()
    # partition-major query/key layout: partition p holds queries {16p+t},
    # keys {32p+c}; the output DMA applies the inverse permutation.
    q3 = q_d.rearrange("(p t) d -> p t d", p=128)
    k3 = k_d.rearrange("(p c) d -> p c d", p=128)
    v3 = v_d.rearrange("(p c) d -> p c d", p=128)
    o3 = o_d.rearrange("(p t) d -> p t d", p=128)

    with tile.TileContext(nc) as tc:
        import contextlib

        ctx = contextlib.ExitStack()
        with ctx:
            big = ctx.enter_context(tc.tile_pool(name="big", bufs=1))
            atp = ctx.enter_context(tc.tile_pool(name="atp", bufs=4))
            qk_ps = ctx.enter_context(tc.tile_pool(name="qk_ps", bufs=2, space="PSUM"))
            mp_ps = ctx.enter_context(tc.tile_pool(name="mp_ps", bufs=1, space="PSUM"))
            mpb_ps = ctx.enter_context(tc.tile_pool(name="mpb_ps", bufs=1, space="PSUM"))
            pv_ps = ctx.enter_context(tc.tile_pool(name="pv_ps", bufs=1, space="PSUM"))

            ident16 = big.tile([128, 128], fp16)
            make_identity(nc, ident16[:])
            ident32 = big.tile([128, 128], f32)
            make_identity(nc, ident32[:])

            q_sb = big.tile([128, QT, D], f32)
            k_sb = big.tile([128, CH, D], f32)
            v_sb = big.tile([128, CH, D], f32)
            q16 = big.tile([128, QT, D], fp16)
            k16 = big.tile([128, CH, D], fp16)
            v_aug = big.tile([128, CH, 65], bf16)
            kT_pack = big.tile([65, CH, 128], fp16)
            qkmov = big.tile([65, QT, 128], fp16)
            M_all = big.tile([128, QT], f32)
            mpp = big.tile([128, QT, 5], f32)  # per-fill max partials
            out_all = big.tile([128, QT, D], f32)
            rZ = big.tile([128, BLK * 4], f32)

            # ---- input DMAs: gpsimd SWDGE casts f32->fp16/bf16 in flight
            nc.gpsimd.dma_start(out=k16[:, 0:16, :], in_=k3[:, 0:16, :])
            nc.gpsimd.dma_start(out=q16[:], in_=q3)
            nc.gpsimd.dma_start(out=k16[:, 16:32, :], in_=k3[:, 16:32, :])
            nc.gpsimd.dma_start(out=v_aug[:, :, 0:64], in_=v3)

            # ---- casts (f32 -> fp16/bf16) + constants, split across engines
            nc.gpsimd.memset(kT_pack[64:65, :, :].bitcast(u32), 0xBC00BC00)

            # ---- PE transposes to build kT_pack rows 0:64 and qkmov rows 0:64
            _tb_rot = [0]

            def transpose_batch(src, n_tiles, dst, dst_off):
                # src: [128, n, 64] fp16; dst rows 0:64, tiles dst_off..
                # rotate psum pools + eviction engine to avoid serialization
                for g in range((n_tiles + 3) // 4):
                    r = _tb_rot[0] = _tb_rot[0] + 1
                    if r % 3 == 0:
                        p_t = mp_ps.tile([128, 2, 512], f32, tag="mp")
                    elif r % 3 == 1:
                        p_t = mpb_ps.tile([128, 1, 512], f32, tag="mpb")
                    else:
                        p_t = qk_ps.tile([128, 2, 512], f32, tag="qk")
                    p16 = p_t[:].bitcast(fp16).rearrange("p a b -> p (a b)")
                    cnt = min(4, n_tiles - g * 4)
                    for i in range(cnt):
                        c = g * 4 + i
                        nc.tensor.transpose(
                            p16[0:64, i * 128 : (i + 1) * 128],
                            src[:, c, :],
                            ident16[:],
                        )
                    (nc.scalar.copy if r % 3 != 0 else nc.vector.tensor_copy)(
                        dst[0:64, dst_off + g * 4 : dst_off + g * 4 + cnt, :],
                        p16[0:64, 0 : cnt * 128].rearrange(
                            "p (c x) -> p c x", x=128
                        ),
                    )

            def kprep_gen():
                for g in range(CH // 4):
                    transpose_batch(
                        k16[:, g * 4 : (g + 1) * 4, :], 4, kT_pack, g * 4
                    )
                    yield

            def qprep_gen(lo, hi):
                for g in range(lo, hi):
                    transpose_batch(
                        q16[:, g * 4 : (g + 1) * 4, :], 4, qkmov, g * 4
                    )
                    yield

            def vprep_gen():
                nc.vector.memset(v_aug[:, :, 64:65], 1.0)
                yield

            def run_interleaved(gens):
                # gens: list of (gen, num, den): advance `num` steps every
                # `den` rounds
                state = [[g, num, den, 0] for g, num, den in gens]
                while state:
                    for ent in list(state):
                        gen, num, den, acc = ent
                        ent[3] = acc = acc + num
                        steps, ent[3] = divmod(acc, den)
                        for _ in range(steps):
                            try:
                                next(gen)
                            except StopIteration:
                                state.remove(ent)
                                break

            # ---- stage generators -------------------------------------
            def mp_stage(bi):
                t0, nt = BLOCKS[bi]
                # max-pass for q-tiles 4b..4b+3: scores [128q, 512k] per mm
                # into alternating psum tiles, reduced on DVE. The M-shuffle
                # for tile t is deferred a couple of fills so the PE transpose
                # never waits on the DVE reduce backlog.
                deferred = []

                def finish_tile(t):
                    nc.vector.reduce_max(M_all[:, t : t + 1], mpp[:, t, :], axis=AX)
                    mt_t = mpb_ps.tile([128, 1, 512], f32, tag="mpb")
                    nc.tensor.transpose(
                        mt_t[0:1, 0, 0:128],
                        M_all[:, t : t + 1],
                        ident32[:],
                    )
                    nc.scalar.copy(qkmov[64:65, t, :], mt_t[0:1, 0, 0:128])

                for ti in range(nt):
                    t = t0 + ti
                    g = 0
                    for fi, nmm in enumerate((2, 1, 2, 1, 2)):
                        if nmm == 2:
                            p_m = mp_ps.tile([128, 2, 512], f32, tag="mp")
                        else:
                            p_m = mpb_ps.tile([128, 1, 512], f32, tag="mpb")
                        for i in range(nmm):
                            nc.tensor.matmul(
                                p_m[:, i, :],
                                qkmov[0:64, t, :],
                                kT_pack[0:64, 4 * g : 4 * g + 4, :],
                                start=True,
                                stop=True,
                            )
                            g += 1
                        nc.vector.reduce_max(
                            mpp[:, t, fi : fi + 1],
                            p_m[:, 0:nmm, :],
                            axis=mybir.AxisListType.XY,
                        )
                        if fi == 2 and deferred:
                            finish_tile(deferred.pop(0))
                        yield
                    deferred.append(t)
                for t in deferred:
                    finish_tile(t)
                    yield

            def block_stage(bi):
                # QK (fp16, with -M row) -> exp -> PV. PV for group g is
                # emitted after QK of group g+1 so the PE never waits on ACT.
                t0, nt = BLOCKS[bi]
                cpt = 8 // nt  # chunks per exp tile (width 1024 cols)
                p_o = pv_ps.tile([128, nt, 65], f32, tag="pv")

                def pv_emit(g, at):
                    for cc in range(cpt):
                        c = g * cpt + cc
                        for j in range(nt):
                            nc.tensor.matmul(
                                p_o[:, j, :],
                                at[:, cc, j * 128 : (j + 1) * 128],
                                v_aug[:, c, :],
                                start=(c == 0 and j == 0),
                                stop=(c == CH - 1 and j == nt - 1),
                            )

                pending = None
                for g in range(CH // cpt):
                    p_s = qk_ps.tile([128, cpt, nt * 128], f32, tag="qk")
                    for cc in range(cpt):
                        c = g * cpt + cc
                        nc.tensor.matmul(
                            p_s[:, cc, :],
                            kT_pack[:, c, :],
                            qkmov[:, t0 : t0 + nt, :],
                            start=True,
                            stop=True,
                        )
                    at = atp.tile([128, cpt, nt * 128], bf16, tag="at")
                    nc.scalar.activation(
                        out=at[:], in_=p_s[:], func=Exp, bias=0.0, scale=SCALE
                    )
                    if pending is not None:
                        pv_emit(*pending)
                    pending = (g, at)
                    yield
                pv_emit(*pending)
                # epilogue: normalize by Z (column 64) and store
                for j in range(nt):
                    r = rZ[:, t0 + j : t0 + j + 1]
                    nc.vector.reciprocal(r, p_o[:, j, 64:65])
                    if (t0 + j) % 2 == 0:
                        nc.scalar.mul(out_all[:, t0 + j, :], p_o[:, j, 0:64], r)
                    else:
                        nc.vector.tensor_scalar_mul(
                            out_all[:, t0 + j, :], p_o[:, j, 0:64], r
                        )
                nc.sync.dma_start(
                    out=o3[:, t0 : t0 + nt, :],
                    in_=out_all[:, t0 : t0 + nt, :],
                )
                yield

            # software pipeline: mp(0); then [mp(b+1) | block(b)] interleaved
            run_interleaved([(qprep_gen(0, 1), 1, 1), (kprep_gen(), 1, 1)])
            run_interleaved(
                [
                    (mp_stage(0), 3, 2),
                    (qprep_gen(1, QT // 4), 1, 2),
                    (vprep_gen(), 1, 4),
                ]
            )
            NB = len(BLOCKS)
            for bi in range(NB):
                _, nt = BLOCKS[bi]
                block_yields = CH // (8 // nt) + 1
                gens = [(block_stage(bi), 1, 1)]
                if bi + 1 < NB:
                    nt_next = BLOCKS[bi + 1][1]
                    mp_yields = 6 * nt_next
                    gens.append((mp_stage(bi + 1), mp_yields, block_yields))
                run_interleaved(gens)

    nc.compile()
    return nc


def kernel(q, k, v):
    if "nc" not in _cached:
        _cached["nc"] = build_program()
    nc = _cached["nc"]
    in_maps = []
    for c in range(NCORES):
        b, h = c // 2, c % 2
        in_maps.append(
            {
                "q": np.ascontiguousarray(q[b, h * NQ : (h + 1) * NQ, :]),
                "k": np.ascontiguousarray(k[b]),
                "v": np.ascontiguousarray(v[b]),
            }
        )
    res = run_bass_kernel_spmd(nc, in_maps, list(range(NCORES)))
    out = np.empty((B, N, D), dtype=np.float32)
    for c in range(NCORES):
        b, h = c // 2, c % 2
        out[b, h * NQ : (h + 1) * NQ, :] = res.results[c]["o"]
    return out



# revision 2
# speedup vs baseline: 1.0025x; 1.0025x over previous
import sys

sys.path.insert(0, "/opt/trn_rl_repo")
import numpy as np

import concourse.bass as bass
import concourse.tile as tile
from concourse import bacc, mybir
from concourse.bass_utils import run_bass_kernel_spmd
from concourse.masks import make_identity

f32 = mybir.dt.float32
bf16 = mybir.dt.bfloat16
fp16 = mybir.dt.float16
u32 = mybir.dt.uint32
fp8 = mybir.dt.float8e4
DR = mybir.MatmulPerfMode.DoubleRow
Exp = mybir.ActivationFunctionType.Exp
AX = mybir.AxisListType.X
MAX = mybir.AluOpType.max
SUBOP = mybir.AluOpType.subtract

B, N, D = 4, 4096, 64
NCORES = 8
NQ = 2048  # queries per core (half a batch)
NK = 4096  # keys per core
QT = NQ // 128  # 16 q-tiles
CH = NK // 128  # 32 k-chunks
BLK = 4  # query blocks of 512 (legacy)
BLOCKS = [(0, 1), (1, 1)] + [(2 + 2 * i, 2) for i in range(6)] + [(14, 1), (15, 1)]  # (tile0, ntiles)
SCALE = 64.0  # sqrt(N)

_cached = {}


def build_program():
    nc = bacc.Bacc("TRN2", target_bir_lowering=False, debug=False, num_devices=NCORES)
    q_d = nc.dram_tensor("q", [NQ, D], f32, kind="ExternalInput").ap()
    k_d = nc.dram_tensor("k", [NK, D], f32, kind="ExternalInput").ap()
    v_d = nc.dram_tensor("v", [NK, D], f32, kind="ExternalInput").ap()
    o_d = nc.dram_tensor("o", [NQ, D], f32, kind="ExternalOutput").ap()
    # partition-major query/key layout: partition p holds queries {16p+t},
    # keys {32p+c}; the output DMA applies the inverse permutation.
    q3 = q_d.rearrange("(p t) d -> p t d", p=128)
    k3 = k_d.rearrange("(p c) d -> p c d", p=128)
    v3 = v_d.rearrange("(p c) d -> p c d", p=128)
    o3 = o_d.rearrange("(p t) d -> p t d", p=128)

    with tile.TileContext(nc) as tc:
        import contextlib

        ctx = contextlib.ExitStack()
        with ctx:
            big = ctx.enter_context(tc.tile_pool(name="big", bufs=1))
            atp = ctx.enter_context(tc.tile_pool(name="atp", bufs=4))
            qk_ps = ctx.enter_context(tc.tile_pool(name="qk_ps", bufs=2, space="PSUM"))
            mp_ps = ctx.enter_context(tc.tile_pool(name="mp_ps", bufs=1, space="PSUM"))
            mpb_ps = ctx.enter_context(tc.tile_pool(name="mpb_ps", bufs=1, space="PSUM"))
            pv_ps = ctx.enter_context(tc.tile_pool(name="pv_ps", bufs=1, space="PSUM"))

            ident16 = big.tile([128, 128], fp16)
            make_identity(nc, ident16[:])
            ident32 = big.tile([128, 128], f32)
            make_identity(nc, ident32[:])

            q_sb = big.tile([128, QT, D], f32)
            k_sb = big.tile([128, CH, D], f32)
            v_sb = big.tile([128, CH, D], f32)
            q16 = big.tile([128, QT, D], fp16)
            k16 = big.tile([128, CH, D], fp16)
            v_aug = big.tile([128, CH, 65], bf16)
            kT_pack = big.tile([65, CH, 128], fp16)
            qkmov = big.tile([65, QT, 128], fp16)
            M_all = big.tile([128, QT], f32)
            mpp = big.tile([128, QT, 5], f32)  # per-fill max partials
            out_all = big.tile([128, QT, D], f32)
            rZ = big.tile([128, BLK * 4], f32)
            q8p = big.tile([64, 2, NQ], fp8)  # planes: (q8, dq)
            k8p = big.tile([64, 2, NK], fp8)  # planes: (k8, k8)

            # ---- input DMAs: gpsimd SWDGE casts f32->fp16/bf16 in flight
            nc.gpsimd.dma_start(out=k16[:, 0:16, :], in_=k3[:, 0:16, :])
            nc.gpsimd.dma_start(out=q16[:], in_=q3)
            nc.gpsimd.dma_start(out=k16[:, 16:32, :], in_=k3[:, 16:32, :])
            nc.gpsimd.dma_start(out=v_aug[:, :, 0:64], in_=v3)

            # ---- casts (f32 -> fp16/bf16) + constants, split across engines
            nc.gpsimd.memset(kT_pack[64:65, :, :].bitcast(u32), 0xBC00BC00)

            # ---- PE transposes to build kT_pack rows 0:64 and qkmov rows 0:64
            _tb_rot = [0]

            def transpose_batch(src, n_tiles, dst, dst_off):
                # src: [128, n, 64] fp16; dst rows 0:64, tiles dst_off..
                # rotate psum pools + eviction engine to avoid serialization
                for g in range((n_tiles + 3) // 4):
                    r = _tb_rot[0] = _tb_rot[0] + 1
                    if r % 3 == 0:
                        p_t = mp_ps.tile([128, 2, 512], f32, tag="mp")
                    elif r % 3 == 1:
                        p_t = mpb_ps.tile([128, 1, 512], f32, tag="mpb")
                    else:
                        p_t = qk_ps.tile([128, 2, 512], f32, tag="qk")
                    p16 = p_t[:].bitcast(fp16).rearrange("p a b -> p (a b)")
                    cnt = min(4, n_tiles - g * 4)
                    for i in range(cnt):
                        c = g * 4 + i
                        nc.tensor.transpose(
                            p16[0:64, i * 128 : (i + 1) * 128],
                            src[:, c, :],
                            ident16[:],
                        )
                    (nc.scalar.copy if r % 3 != 0 else nc.vector.tensor_copy)(
                        dst[0:64, dst_off + g * 4 : dst_off + g * 4 + cnt, :],
                        p16[0:64, 0 : cnt * 128].rearrange(
                            "p (c x) -> p c x", x=128
                        ),
                    )

            def kprep_gen():
                for g in range(CH // 4):
                    transpose_batch(
                        k16[:, g * 4 : (g + 1) * 4, :], 4, kT_pack, g * 4
                    )
                    yield

            def qprep_gen(lo, hi):
                for g in range(lo, hi):
                    transpose_batch(
                        q16[:, g * 4 : (g + 1) * 4, :], 4, qkmov, g * 4
                    )
                    sl = slice(g * 512, (g + 1) * 512)
                    qsl = qkmov[0:64, g * 4 : (g + 1) * 4, :].rearrange(
                        "p t d -> p (t d)"
                    )
                    nc.gpsimd.dma_start(out=q8p[:, 0, sl], in_=qsl)
                    nc.gpsimd.tensor_tensor(
                        out=q8p[:, 1, sl], in0=qsl, in1=q8p[:, 0, sl], op=SUBOP
                    )
                    yield

            def vprep_gen():
                nc.vector.memset(v_aug[:, :, 64:65], 1.0)
                yield

            def run_interleaved(gens):
                # gens: list of (gen, num, den): advance `num` steps every
                # `den` rounds
                state = [[g, num, den, 0] for g, num, den in gens]
                while state:
                    for ent in list(state):
                        gen, num, den, acc = ent
                        ent[3] = acc = acc + num
                        steps, ent[3] = divmod(acc, den)
                        for _ in range(steps):
                            try:
                                next(gen)
                            except StopIteration:
                                state.remove(ent)
                                break

            # ---- stage generators -------------------------------------
            def mp_stage(bi):
                t0, nt = BLOCKS[bi]
                # max-pass for q-tiles 4b..4b+3: scores [128q, 512k] per mm
                # into alternating psum tiles, reduced on DVE. The M-shuffle
                # for tile t is deferred a couple of fills so the PE transpose
                # never waits on the DVE reduce backlog.
                deferred = []

                def finish_tile(t):
                    nc.vector.reduce_max(M_all[:, t : t + 1], mpp[:, t, :], axis=AX)
                    mt_t = mpb_ps.tile([128, 1, 512], f32, tag="mpb")
                    nc.tensor.transpose(
                        mt_t[0:1, 0, 0:128],
                        M_all[:, t : t + 1],
                        ident32[:],
                    )
                    nc.scalar.copy(qkmov[64:65, t, :], mt_t[0:1, 0, 0:128])

                for ti in range(nt):
                    t = t0 + ti
                    g = 0
                    for fi, nmm in enumerate((2, 1, 2, 1, 2)):
                        if nmm == 2:
                            p_m = mp_ps.tile([128, 2, 512], f32, tag="mp")
                        else:
                            p_m = mpb_ps.tile([128, 1, 512], f32, tag="mpb")
                        for i in range(nmm):
                            if t < 2:
                                nc.tensor.matmul(
                                    p_m[:, i, :],
                                    qkmov[0:64, t, :],
                                    kT_pack[0:64, 4 * g : 4 * g + 4, :],
                                    start=True,
                                    stop=True,
                                )
                            else:
                                nc.tensor.matmul(
                                    p_m[:, i, :],
                                    q8p[:, :, t * 128 : (t + 1) * 128],
                                    k8p[:, :, g * 512 : (g + 1) * 512],
                                    start=True,
                                    stop=True,
                                    perf_mode=DR,
                                )
                            g += 1
                        nc.vector.reduce_max(
                            mpp[:, t, fi : fi + 1],
                            p_m[:, 0:nmm, :],
                            axis=mybir.AxisListType.XY,
                        )
                        if fi == 2 and deferred:
                            finish_tile(deferred.pop(0))
                        yield
                    deferred.append(t)
                for t in deferred:
                    finish_tile(t)
                    yield

            def block_stage(bi):
                # QK (fp16, with -M row) -> exp -> PV. PV for group g is
                # emitted after QK of group g+1 so the PE never waits on ACT.
                t0, nt = BLOCKS[bi]
                cpt = 8 // nt  # chunks per exp tile (width 1024 cols)
                p_o = pv_ps.tile([128, nt, 65], f32, tag="pv")

                def pv_emit(g, at):
                    for cc in range(cpt):
                        c = g * cpt + cc
                        for j in range(nt):
                            nc.tensor.matmul(
                                p_o[:, j, :],
                                at[:, cc, j * 128 : (j + 1) * 128],
                                v_aug[:, c, :],
                                start=(c == 0 and j == 0),
                                stop=(c == CH - 1 and j == nt - 1),
                            )

                pending = None
                for g in range(CH // cpt):
                    p_s = qk_ps.tile([128, cpt, nt * 128], f32, tag="qk")
                    for cc in range(cpt):
                        c = g * cpt + cc
                        nc.tensor.matmul(
                            p_s[:, cc, :],
                            kT_pack[:, c, :],
                            qkmov[:, t0 : t0 + nt, :],
                            start=True,
                            stop=True,
                        )
                    at = atp.tile([128, cpt, nt * 128], bf16, tag="at")
                    nc.scalar.activation(
                        out=at[:], in_=p_s[:], func=Exp, bias=0.0, scale=SCALE
                    )
                    if pending is not None:
                        pv_emit(*pending)
                    pending = (g, at)
                    yield
                pv_emit(*pending)
                # epilogue: normalize by Z (column 64) and store
                for j in range(nt):
                    r = rZ[:, t0 + j : t0 + j + 1]
                    nc.vector.reciprocal(r, p_o[:, j, 64:65])
                    if (t0 + j) % 2 == 0:
                        nc.scalar.mul(out_all[:, t0 + j, :], p_o[:, j, 0:64], r)
                    else:
                        nc.vector.tensor_scalar_mul(
                            out_all[:, t0 + j, :], p_o[:, j, 0:64], r
                        )
                nc.sync.dma_start(
                    out=o3[:, t0 : t0 + nt, :],
                    in_=out_all[:, t0 : t0 + nt, :],
                )
                yield

            # software pipeline: mp(0); then [mp(b+1) | block(b)] interleaved
            run_interleaved([(qprep_gen(0, 1), 1, 1), (kprep_gen(), 1, 1)])
            qk4 = qkmov[0:64, :, :].rearrange("p t d -> p (t d)")
            kt4 = kT_pack[0:64, :, :].rearrange("p t d -> p (t d)")
            nc.gpsimd.dma_start(out=k8p[:, 0, :], in_=kt4)
            nc.gpsimd.dma_start(out=k8p[:, 1, :], in_=kt4)
            run_interleaved(
                [
                    (mp_stage(0), 3, 2),
                    (qprep_gen(1, QT // 4), 1, 2),
                    (vprep_gen(), 1, 4),
                ]
            )
            NB = len(BLOCKS)
            for bi in range(NB):
                _, nt = BLOCKS[bi]
                block_yields = CH // (8 // nt) + 1
                gens = [(block_stage(bi), 1, 1)]
                if bi + 1 < NB:
                    nt_next = BLOCKS[bi + 1][1]
                    mp_yields = 4 * 6 * nt_next
                    gens.append((mp_stage(bi + 1), mp_yields, block_yields))
                run_interleaved(gens)

    nc.compile()
    return nc


def kernel(q, k, v):
    if "nc" not in _cached:
        _cached["nc"] = build_program()
    nc = _cached["nc"]
    in_maps = []
    for c in range(NCORES):
        b, h = c // 2, c % 2
        in_maps.append(
            {
                "q": np.ascontiguousarray(q[b, h * NQ : (h + 1) * NQ, :]),
                "k": np.ascontiguousarray(k[b]),
                "v": np.ascontiguousarray(v[b]),
            }
        )
    res = run_bass_kernel_spmd(nc, in_maps, list(range(NCORES)))
    out = np.empty((B, N, D), dtype=np.float32)
    for c in range(NCORES):
        b, h = c // 2, c % 2
        out[b, h * NQ : (h + 1) * NQ, :] = res.results[c]["o"]
    return out



# revision 3
# speedup vs baseline: 1.0160x; 1.0135x over previous
import sys

sys.path.insert(0, "/opt/trn_rl_repo")
import numpy as np

import concourse.bass as bass
import concourse.tile as tile
from concourse import bacc, mybir
from concourse.bass_utils import run_bass_kernel_spmd
from concourse.masks import make_identity

f32 = mybir.dt.float32
bf16 = mybir.dt.bfloat16
fp16 = mybir.dt.float16
u32 = mybir.dt.uint32
fp8 = mybir.dt.float8e4
DR = mybir.MatmulPerfMode.DoubleRow
Exp = mybir.ActivationFunctionType.Exp
AX = mybir.AxisListType.X
MAX = mybir.AluOpType.max
SUBOP = mybir.AluOpType.subtract

B, N, D = 4, 4096, 64
NCORES = 8
NQ = 2048  # queries per core (half a batch)
NK = 4096  # keys per core
QT = NQ // 128  # 16 q-tiles
CH = NK // 128  # 32 k-chunks
BLK = 4  # query blocks of 512 (legacy)
BLOCKS = [(0, 1), (1, 1)] + [(2 + 2 * i, 2) for i in range(6)] + [(14, 1), (15, 1)]  # (tile0, ntiles)
SCALE = 64.0  # sqrt(N)

_cached = {}


def build_program():
    nc = bacc.Bacc("TRN2", target_bir_lowering=False, debug=False, num_devices=NCORES)
    q_d = nc.dram_tensor("q", [NQ, D], f32, kind="ExternalInput").ap()
    k_d = nc.dram_tensor("k", [NK, D], f32, kind="ExternalInput").ap()
    v_d = nc.dram_tensor("v", [NK, D], f32, kind="ExternalInput").ap()
    o_d = nc.dram_tensor("o", [NQ, D], f32, kind="ExternalOutput").ap()
    # partition-major query/key layout: partition p holds queries {16p+t},
    # keys {32p+c}; the output DMA applies the inverse permutation.
    q3 = q_d.rearrange("(p t) d -> p t d", p=128)
    k3 = k_d.rearrange("(p c) d -> p c d", p=128)
    v3 = v_d.rearrange("(p c) d -> p c d", p=128)
    o3 = o_d.rearrange("(p t) d -> p t d", p=128)

    with tile.TileContext(nc) as tc:
        import contextlib

        ctx = contextlib.ExitStack()
        with ctx:
            big = ctx.enter_context(tc.tile_pool(name="big", bufs=1))
            atp = ctx.enter_context(tc.tile_pool(name="atp", bufs=4))
            qk_ps = ctx.enter_context(tc.tile_pool(name="qk_ps", bufs=2, space="PSUM"))
            mp_ps = ctx.enter_context(tc.tile_pool(name="mp_ps", bufs=1, space="PSUM"))
            mpb_ps = ctx.enter_context(tc.tile_pool(name="mpb_ps", bufs=1, space="PSUM"))
            pv_ps = ctx.enter_context(tc.tile_pool(name="pv_ps", bufs=1, space="PSUM"))

            ident16 = big.tile([128, 128], fp16)
            make_identity(nc, ident16[:])
            ident32 = big.tile([128, 128], f32)
            make_identity(nc, ident32[:])

            q_sb = big.tile([128, QT, D], f32)
            k_sb = big.tile([128, CH, D], f32)
            v_sb = big.tile([128, CH, D], f32)
            q16 = big.tile([128, QT, D], fp16)
            k16 = big.tile([128, CH, D], fp16)
            v_aug = big.tile([128, CH, 65], bf16)
            kT_pack = big.tile([65, CH, 128], fp16)
            qkmov = big.tile([65, QT, 128], fp16)
            M_all = big.tile([128, QT], f32)
            mpp = big.tile([128, QT, 5], f32)  # per-fill max partials
            out_all = big.tile([128, QT, D], f32)
            rZ = big.tile([128, BLK * 4], f32)
            q8p = big.tile([64, 2, NQ], fp8)  # planes: (q8, dq)
            k8p = big.tile([64, 2, NK], fp8)  # planes: (k8, k8)

            # ---- input DMAs: gpsimd SWDGE casts f32->fp16/bf16 in flight
            nc.gpsimd.dma_start(out=k16[:, 0:16, :], in_=k3[:, 0:16, :])
            nc.gpsimd.dma_start(out=q16[:], in_=q3)
            nc.gpsimd.dma_start(out=k16[:, 16:32, :], in_=k3[:, 16:32, :])
            nc.gpsimd.dma_start(out=v_aug[:, :, 0:64], in_=v3)

            # ---- casts (f32 -> fp16/bf16) + constants, split across engines
            nc.gpsimd.memset(kT_pack[64:65, :, :].bitcast(u32), 0xBC00BC00)

            # ---- PE transposes to build kT_pack rows 0:64 and qkmov rows 0:64
            _tb_rot = [0]

            def transpose_batch(src, n_tiles, dst, dst_off):
                # src: [128, n, 64] fp16; dst rows 0:64, tiles dst_off..
                # rotate psum pools + eviction engine to avoid serialization
                for g in range((n_tiles + 3) // 4):
                    r = _tb_rot[0] = _tb_rot[0] + 1
                    if r % 3 == 0:
                        p_t = mp_ps.tile([128, 2, 512], f32, tag="mp")
                    elif r % 3 == 1:
                        p_t = mpb_ps.tile([128, 1, 512], f32, tag="mpb")
                    else:
                        p_t = qk_ps.tile([128, 2, 512], f32, tag="qk")
                    p16 = p_t[:].bitcast(fp16).rearrange("p a b -> p (a b)")
                    cnt = min(4, n_tiles - g * 4)
                    for i in range(cnt):
                        c = g * 4 + i
                        nc.tensor.transpose(
                            p16[0:64, i * 128 : (i + 1) * 128],
                            src[:, c, :],
                            ident16[:],
                        )
                    (nc.scalar.copy if r % 3 != 0 else nc.vector.tensor_copy)(
                        dst[0:64, dst_off + g * 4 : dst_off + g * 4 + cnt, :],
                        p16[0:64, 0 : cnt * 128].rearrange(
                            "p (c x) -> p c x", x=128
                        ),
                    )

            def kprep_gen():
                for g in range(CH // 4):
                    transpose_batch(
                        k16[:, g * 4 : (g + 1) * 4, :], 4, kT_pack, g * 4
                    )
                    yield

            def qprep_gen(lo, hi):
                for g in range(lo, hi):
                    transpose_batch(
                        q16[:, g * 4 : (g + 1) * 4, :], 4, qkmov, g * 4
                    )
                    sl = slice(g * 512, (g + 1) * 512)
                    qsl = qkmov[0:64, g * 4 : (g + 1) * 4, :].rearrange(
                        "p t d -> p (t d)"
                    )
                    nc.gpsimd.dma_start(out=q8p[:, 0, sl], in_=qsl)
                    nc.gpsimd.tensor_tensor(
                        out=q8p[:, 1, sl], in0=qsl, in1=q8p[:, 0, sl], op=SUBOP
                    )
                    yield

            def vprep_gen():
                nc.vector.memset(v_aug[:, :, 64:65], 1.0)
                yield

            def run_interleaved(gens):
                # gens: list of (gen, num, den): advance `num` steps every
                # `den` rounds
                state = [[g, num, den, 0] for g, num, den in gens]
                while state:
                    for ent in list(state):
                        gen, num, den, acc = ent
                        ent[3] = acc = acc + num
                        steps, ent[3] = divmod(acc, den)
                        for _ in range(steps):
                            try:
                                next(gen)
                            except StopIteration:
                                state.remove(ent)
                                break

            # ---- stage generators -------------------------------------
            def mp_stage(bi):
                t0, nt = BLOCKS[bi]
                # max-pass for q-tiles 4b..4b+3: scores [128q, 512k] per mm
                # into alternating psum tiles, reduced on DVE. The M-shuffle
                # for tile t is deferred a couple of fills so the PE transpose
                # never waits on the DVE reduce backlog.
                deferred = []

                def finish_tile(t):
                    nc.vector.reduce_max(M_all[:, t : t + 1], mpp[:, t, :], axis=AX)
                    mt_t = mpb_ps.tile([128, 1, 512], f32, tag="mpb")
                    nc.tensor.transpose(
                        mt_t[0:1, 0, 0:128],
                        M_all[:, t : t + 1],
                        ident32[:],
                    )
                    nc.scalar.copy(qkmov[64:65, t, :], mt_t[0:1, 0, 0:128])

                for ti in range(nt):
                    t = t0 + ti
                    g = 0
                    for fi, nmm in enumerate((2, 1, 2, 1, 2)):
                        if nmm == 2:
                            p_m = mp_ps.tile([128, 2, 512], f32, tag="mp")
                        else:
                            p_m = mpb_ps.tile([128, 1, 512], f32, tag="mpb")
                        for i in range(nmm):
                            if t < 2:
                                nc.tensor.matmul(
                                    p_m[:, i, :],
                                    qkmov[0:64, t, :],
                                    kT_pack[0:64, 4 * g : 4 * g + 4, :],
                                    start=True,
                                    stop=True,
                                )
                            else:
                                nc.tensor.matmul(
                                    p_m[:, i, :],
                                    q8p[:, :, t * 128 : (t + 1) * 128],
                                    k8p[:, :, g * 512 : (g + 1) * 512],
                                    start=True,
                                    stop=True,
                                    perf_mode=DR,
                                )
                            g += 1
                        nc.vector.reduce_max(
                            mpp[:, t, fi : fi + 1],
                            p_m[:, 0:nmm, :],
                            axis=mybir.AxisListType.XY,
                        )
                        if fi == 2 and deferred:
                            finish_tile(deferred.pop(0))
                        yield
                    deferred.append(t)
                for t in deferred:
                    finish_tile(t)
                    yield

            def block_stage(bi):
                # QK (fp16, with -M row) -> exp -> PV. PV for group g is
                # emitted after QK of group g+1 so the PE never waits on ACT.
                t0, nt = BLOCKS[bi]
                cpt = 8 // nt  # chunks per exp tile (width 1024 cols)
                p_o = pv_ps.tile([128, nt, 65], f32, tag="pv")

                def pv_emit(g, at):
                    for cc in range(cpt):
                        c = g * cpt + cc
                        for j in range(nt):
                            nc.tensor.matmul(
                                p_o[:, j, :],
                                at[:, cc, j * 128 : (j + 1) * 128],
                                v_aug[:, c, :],
                                start=(c == 0 and j == 0),
                                stop=(c == CH - 1 and j == nt - 1),
                            )

                pending = None
                for g in range(CH // cpt):
                    p_s = qk_ps.tile([128, cpt, nt * 128], f32, tag="qk")
                    for cc in range(cpt):
                        c = g * cpt + cc
                        nc.tensor.matmul(
                            p_s[:, cc, :],
                            kT_pack[:, c, :],
                            qkmov[:, t0 : t0 + nt, :],
                            start=True,
                            stop=True,
                        )
                    at = atp.tile([128, cpt, nt * 128], bf16, tag="at")
                    nc.scalar.activation(
                        out=at[:], in_=p_s[:], func=Exp, bias=0.0, scale=SCALE
                    )
                    if pending is not None:
                        pv_emit(*pending)
                    pending = (g, at)
                    yield
                pv_emit(*pending)
                # epilogue: normalize by Z (column 64) and store
                for j in range(nt):
                    r = rZ[:, t0 + j : t0 + j + 1]
                    nc.vector.reciprocal(r, p_o[:, j, 64:65])
                    if (t0 + j) % 2 == 0:
                        nc.scalar.mul(out_all[:, t0 + j, :], p_o[:, j, 0:64], r)
                    else:
                        nc.vector.tensor_scalar_mul(
                            out_all[:, t0 + j, :], p_o[:, j, 0:64], r
                        )
                nc.sync.dma_start(
                    out=o3[:, t0 : t0 + nt, :],
                    in_=out_all[:, t0 : t0 + nt, :],
                )
                yield

            # software pipeline: mp(0); then [mp(b+1) | block(b)] interleaved
            run_interleaved([(qprep_gen(0, 1), 1, 1), (kprep_gen(), 1, 1)])
            qk4 = qkmov[0:64, :, :].rearrange("p t d -> p (t d)")
            kt4 = kT_pack[0:64, :, :].rearrange("p t d -> p (t d)")
            nc.gpsimd.dma_start(out=k8p[:, 0, :], in_=kt4)
            nc.gpsimd.dma_start(out=k8p[:, 1, :], in_=kt4)
            run_interleaved(
                [
                    (mp_stage(0), 2, 1),
                    (qprep_gen(1, QT // 4), 1, 2),
                    (vprep_gen(), 1, 4),
                ]
            )
            NB = len(BLOCKS)
            for bi in range(NB):
                _, nt = BLOCKS[bi]
                block_yields = CH // (8 // nt) + 1
                gens = [(block_stage(bi), 1, 1)]
                if bi + 1 < NB:
                    nt_next = BLOCKS[bi + 1][1]
                    mp_yields = 4 * 6 * nt_next
                    gens.append((mp_stage(bi + 1), mp_yields, block_yields))
                run_interleaved(gens)

    nc.compile()
    return nc


def kernel(q, k, v):
    if "nc" not in _cached:
        _cached["nc"] = build_program()
    nc = _cached["nc"]
    in_maps = []
    for c in range(NCORES):
        b, h = c // 2, c % 2
        in_maps.append(
            {
                "q": np.ascontiguousarray(q[b, h * NQ : (h + 1) * NQ, :]),
                "k": np.ascontiguousarray(k[b]),
                "v": np.ascontiguousarray(v[b]),
            }
        )
    res = run_bass_kernel_spmd(nc, in_maps, list(range(NCORES)))
    out = np.empty((B, N, D), dtype=np.float32)
    for c in range(NCORES):
        b, h = c // 2, c % 2
        out[b, h * NQ : (h + 1) * NQ, :] = res.results[c]["o"]
    return out

